# revision 45
# baseline (speedup 1.0000x reference)
"""BiMamba2D (VMamba-style 4-direction selective scan) Trainium2 Bass kernel.

Sharding: 8 cores = 4 batches x 2 scan layouts (hw / wh).  The wh layout is
realized by host-transposing the input image (and swapping the conv kernel's
spatial taps), so every core runs the same SPMD program.  Each core computes
both time directions (forward + reversed APs) of its layout and emits a
partial (L, 96) output; the host sums partials (gating and the output
projection are linear across the four direction contributions).

Scan-state layout: d-blocks of 8 channels x 16 states = 128 partitions
(row p of a d-block tile holds channel db*8 + p//16, state p%16).  The
recurrence runs as one tensor_tensor_scan per (d-block, time-chunk).
The 192 inner channels are split as 128 + 64 rows so every partition
offset is quad-aligned (0/64), which the engines require.

Performance notes (2.49 ms -> 0.98 ms on HW):
- All matmul operands are bf16 (fp32 matmuls run as 2 half-rate PE passes;
  bf16 is 1 full-rate pass).  Weights are host-transposed so every DMA is a
  dense row read (strided 2-byte gathers cost ~30k descriptors otherwise).
- delta/du are direction-independent: computed once (not per direction) in
  phase 3, bf16.  Exp/Ln are batched per chunk-pair so the activation table
  reloads a handful of times instead of per chunk (1.3 us per reload).
- delta is replicated 8->128 rows by SBUF->SBUF DMA broadcast, issued one
  1024-col block ahead; its consumer (scalar Exp) has slack to absorb DMA
  jitter.  du replication stays on the PE (DMA can't feed both: the 16x
  write amplification saturates the 8 HWDGE queues).
- The reversed direction is realized by giving tensor_tensor_scan reversed
  APs (step -1 on data0/data1/out); everything upstream and downstream
  stays in forward order with aligned fast DVE modes.
- h-state carry chains chunk-to-chunk via per-j h tiles read directly by the
  next scan (a scalar-engine carry copy adds a cross-engine hop that stalls
  the DVE).
- o = h*C runs on gpsimd for even j and the DVE for odd j: the split breaks
  a DVE<->gpsimd<->PE(psY accumulate) semaphore convoy that otherwise
  stalls ~20% of scans.
- Phases 1-3 (transpose, in-proj, conv, projections) are emitted interleaved
  per chunk with coexisting PSUM pools (8 banks exactly), and x/ident DMAs
  are priority-hoisted, so the scan phase starts at ~175 us instead of ~205.
- The DVE is the end-to-end bottleneck: scans are ~2.2 cyc/elem (feedback
  bubble), dBu is 1x (PSUM operand), ~96% DVE occupancy in the scan window.
"""

import os
import sys
from contextlib import ExitStack

import numpy as np

for _p in ("/opt/trn_rl_repo",):
    if _p not in sys.path and os.path.isdir(_p):
        sys.path.append(_p)

import concourse.bass as bass
import concourse.tile as tile
from concourse import bacc, mybir

F32 = mybir.dt.float32
F32R = mybir.dt.float32r
BF16 = mybir.dt.bfloat16
AL = mybir.AluOpType
AF = mybir.ActivationFunctionType

# Problem constants
B, H, W, CM = 4, 64, 64, 96
L = H * W  # 4096
D = 192  # d_inner
N = 16  # d_state
RK = 6  # dt_rank
TC = 512  # time-chunk
NCH = L // TC  # 8
NDB = D // 8  # 24 d-blocks
NG = 3  # groups of 64 channels
GDB = NDB // NG  # 8 d-blocks per group
HS = [128, 64]  # d_inner row split
HOF = [0, 128]  # absolute channel offset per half
# group -> (half index, row offset within half)
GMAP = [(0, 0), (0, 64), (1, 0)]
WP = W + 2  # padded row stride for conv


def _rev(ap):
    """Reverse an AP along its last (free) dim."""
    return ap[:, ::-1]


def build_kernel(ctx: ExitStack, tc: "tile.TileContext", io: dict):
    nc = tc.nc


    # ---------------- weight / constant loads ----------------
    wpool = ctx.enter_context(tc.tile_pool(name="wpool", bufs=1))

    w_int = wpool.tile([96, 384], BF16, name="w_int")
    nc.sync.dma_start(w_int[:], io["w_in"][:])

    # B/C projections with 16->128 row replication folded in (host-tiled),
    # and the dt projection folded through x_proj (host-matmul'd).
    xpb_t, xpc_t, dtw_t = [], [], []
    for hh in range(2):
        hsl = slice(HOF[hh], HOF[hh] + HS[hh])
        t = wpool.tile([HS[hh], 128], BF16, name=f"xpb_t{hh}")
        nc.sync.dma_start(t[:], io["xpb_w"][hsl, :])
        xpb_t.append(t)
        t = wpool.tile([HS[hh], 128], BF16, name=f"xpc_t{hh}")
        nc.sync.dma_start(t[:], io["xpc_w"][hsl, :])
        xpc_t.append(t)
        t = wpool.tile([HS[hh], 192], BF16, name=f"dtw_t{hh}")
        nc.sync.dma_start(t[:], io["dtw_full"][hsl, :])
        dtw_t.append(t)

    wout_t = []
    for hh in range(2):
        t = wpool.tile([HS[hh], 96], BF16, name=f"wout_t{hh}")
        nc.sync.dma_start(
            t[:], io["w_out"][HOF[hh] : HOF[hh] + HS[hh], :]
        )
        wout_t.append(t)

    def vec_col(name):
        tiles = []
        for hh in range(2):
            t = wpool.tile([HS[hh], 1], F32, name=f"{name}{hh}")
            nc.sync.dma_start(
                t[:],
                io[name][HOF[hh] : HOF[hh] + HS[hh]].rearrange("(p one) -> p one", one=1),
            )
            tiles.append(t)
        return tiles

    dtb = vec_col("dt_proj_b")
    convb = vec_col("conv_b")
    d2 = vec_col("d2")

    a_dn = wpool.tile([128, NDB], F32, name="a_dn")
    nc.sync.dma_start(a_dn[:], io["a_dn"][:])
    # r64 rows are duplicated (0..63 == 64..127) so the lhsT slice can sit
    # at the same base partition as its rhs (a group-base requirement).
    r64 = []  # [j]: [128, 128]; rows k: (k%64 == j*8 + p//16)
    rt64 = []  # [j]: [128, 64] n-contraction lhsT into rows j*8..j*8+8
    for j in range(GDB):
        t = wpool.tile([128, 128], BF16, name=f"r64_{j}")
        nc.sync.dma_start(t[:], io["r64"][j])
        r64.append(t)
        t2 = wpool.tile([128, 64], BF16, name=f"rt64_{j}")
        nc.sync.dma_start(t2[:], io["rt64"][j])
        rt64.append(t2)
    ident = wpool.tile([128, 128], F32, name="ident")
    with tc.high_priority():
        nc.sync.dma_start(ident[:], io["ident"][:])

    # ---------------- persistent big buffers ----------------
    ppool = ctx.enter_context(tc.tile_pool(name="persist", bufs=1))
    xT = ppool.tile([96, L], BF16, name="xT")  # x transposed (ch, t)
    xc = [ppool.tile([HS[hh], L], BF16, name=f"xc{hh}") for hh in range(2)]
    y_sb = [ppool.tile([HS[hh], L], F32, name=f"y{hh}") for hh in range(2)]
    b_rep = ppool.tile([128, L], BF16, name="b_rep")
    c_rep = ppool.tile([128, L], BF16, name="c_rep")

    # del/du/e1 + phase-3 PSUM live past the conv, so enter them below the
    # conv-scoped pools on the pool stack.
    dpool = ctx.enter_context(tc.tile_pool(name="dpool", bufs=1))
    del_sb = [dpool.tile([HS[hh], L], BF16, name=f"del{hh}") for hh in range(2)]
    du_sb = [dpool.tile([HS[hh], L], BF16, name=f"du{hh}") for hh in range(2)]
    p3ps = ctx.enter_context(tc.tile_pool(name="p3ps", bufs=2, space="PSUM"))
    e1pool = ctx.enter_context(tc.tile_pool(name="e1pool", bufs=1))
    e1_sb = [e1pool.tile([HS[hh], L], BF16, name=f"e1_{hh}") for hh in range(2)]

    # ================= phase 1: transpose x + input projection =================
    with (
        tc.tile_pool(name="padpool", bufs=1) as padpool,
        tc.tile_pool(name="cwpool", bufs=1) as cwpool,
    ):
        # conv weights: lhsT [d_in HS[ih], d_out HS[oh]] per (ih, oh, kh, kw)
        cw = {}
        for ih in range(2):
            for oh in range(2):
                for kh in range(3):
                    for kw in range(3):
                        t = cwpool.tile([HS[ih], HS[oh]], BF16, name=f"cw{ih}{oh}{kh}{kw}")
                        src = io["conv_w"][
                            kh,
                            kw,
                            HOF[ih] : HOF[ih] + HS[ih],
                            HOF[oh] : HOF[oh] + HS[oh],
                        ]
                        nc.sync.dma_start(t[:], src)
                        cw[(ih, oh, kh, kw)] = t

        xp_pad = [
            padpool.tile([HS[hh], (H + 2) * WP], BF16, name=f"xp_pad{hh}")
            for hh in range(2)
        ]
        for hh in range(2):
            nc.gpsimd.memset(xp_pad[hh][:], 0.0)

        with (
            tc.tile_pool(name="p1sb", bufs=3) as p1sb,
            tc.tile_pool(name="p1ps", bufs=2, space="PSUM") as p1ps,
        ):
            # x tiles + transposes are the critical path at kernel start;
            # hoist them above the (deferred-use) weight DMAs.
            with tc.high_priority():
                for m in range(L // 128):
                    xt = p1sb.tile([128, 96], F32, tag="xt")
                    nc.sync.dma_start(xt[:], io["x"][m * 128 : (m + 1) * 128, :])
                    ps_t = p1ps.tile([96, 128], F32, tag="ps_t")
                    nc.tensor.transpose(ps_t[:], xt[:], ident[:])
                    nc.scalar.copy(xT[:, m * 128 : (m + 1) * 128], ps_t[:])

            for ch in range(NCH):
                tsl = slice(ch * TC, (ch + 1) * TC)
                for oh in range(2):
                    ps = p1ps.tile([HS[oh], TC], F32, tag=f"ps_ip{oh}", bufs=1)
                    nc.tensor.matmul(
                        ps[:],
                        w_int[:, HOF[oh] : HOF[oh] + HS[oh]],
                        xT[:, tsl],
                        start=True,
                        stop=True,
                    )
                    # write into padded conv buffer rows [ch*8+1..ch*8+8], cols 1..64
                    dst = (
                        xp_pad[oh][:]
                        .rearrange("p (h w) -> p h w", w=WP)[
                            :, ch * 8 + 1 : ch * 8 + 9, 1 : W + 1
                        ]
                    )
                    nc.scalar.copy(dst, ps[:])

        # ========== phase 2+3 interleaved per chunk pair: 3x3 conv ==========
        # + bias/silu, then immediately B/C/dt projections and delta for the
        # same chunk, so the scan phase can start long before the last conv
        # chunk finishes.  Exp/Ln batched per pair to limit ACT table reloads.
        with tc.tile_pool(name="p2ps", bufs=2, space="PSUM") as p2ps:
            for pb in range(0, NCH, 2):
                for ch in (pb, pb + 1):
                    tsl = slice(ch * TC, (ch + 1) * TC)
                    for oh in range(2):
                        ps = p2ps.tile([HS[oh], TC], F32, tag=f"ps_cv{oh}")
                        first = True
                        for ih in range(2):
                            for kh in range(3):
                                for kw in range(3):
                                    rhs = (
                                        xp_pad[ih][:]
                                        .rearrange("p (h w) -> p h w", w=WP)[
                                            :, ch * 8 + kh : ch * 8 + kh + 8, kw : kw + W
                                        ]
                                    )
                                    last = ih == 1 and kh == 2 and kw == 2
                                    nc.tensor.matmul(
                                        ps[:],
                                        cw[(ih, oh, kh, kw)][:],
                                        rhs,
                                        start=first,
                                        stop=last,
                                    )
                                    first = False
                        nc.scalar.activation(
                            xc[oh][:, tsl], ps[:], AF.Silu, bias=convb[oh][:, 0:1]
                        )
                    ps_b = p3ps.tile([128, TC], F32, tag="ps_bc")
                    nc.tensor.matmul(ps_b[:], xpb_t[0][:], xc[0][:, tsl], start=True, stop=False)
                    nc.tensor.matmul(ps_b[:], xpb_t[1][:], xc[1][:, tsl], start=False, stop=True)
                    nc.scalar.copy(b_rep[:, tsl], ps_b[:])
                    ps_c = p3ps.tile([128, TC], F32, tag="ps_bc")
                    nc.tensor.matmul(ps_c[:], xpc_t[0][:], xc[0][:, tsl], start=True, stop=False)
                    nc.tensor.matmul(ps_c[:], xpc_t[1][:], xc[1][:, tsl], start=False, stop=True)
                    nc.scalar.copy(c_rep[:, tsl], ps_c[:])
                    for hh in range(2):
                        hsl = slice(HOF[hh], HOF[hh] + HS[hh])
                        ps_dt = p3ps.tile([HS[hh], TC], F32, tag=f"ps_dt{hh}", bufs=1)
                        nc.tensor.matmul(ps_dt[:], dtw_t[0][:, hsl], xc[0][:, tsl], start=True, stop=False)
                        nc.tensor.matmul(ps_dt[:], dtw_t[1][:, hsl], xc[1][:, tsl], start=False, stop=True)
                        nc.scalar.activation(
                            e1_sb[hh][:, tsl], ps_dt[:], AF.Exp, bias=dtb[hh][:, 0:1]
                        )
                for ch in (pb, pb + 1):
                    tsl = slice(ch * TC, (ch + 1) * TC)
                    for hh in range(2):
                        nc.scalar.activation(
                            del_sb[hh][:, tsl], e1_sb[hh][:, tsl], AF.Ln, bias=1.0
                        )
                        nc.vector.tensor_tensor(
                            du_sb[hh][:, tsl], del_sb[hh][:, tsl], xc[hh][:, tsl], AL.mult
                        )

    # ================= phase 4: selective scan (fwd + rev) =================
    # Everything is kept in forward (data) order; the time-reversed direction
    # is realized purely by giving tensor_tensor_scan reversed APs, so its
    # output h lands back in data order.  du is replicated 64->128 partitions
    # by an SBUF->SBUF DMA broadcast (8 src rows x16), which keeps dBu's
    # operands in SBUF/bf16 (fast DVE mode) and frees PE/PSUM.
    with (
        tc.tile_pool(name="scps", bufs=3, space="PSUM") as scps,
        tc.tile_pool(name="scpsy", bufs=2, space="PSUM") as scpsy,
        tc.tile_pool(name="scsb", bufs=2) as scsb,
        tc.tile_pool(name="hpool", bufs=1) as hpool,
    ):
        for rev in (0, 1):
            for g in range(NG):
                hh, gr0 = GMAP[g]
                h_prev = {}
                gp = slice(gr0, gr0 + 64)  # group's partition slice

                # delta replicated 8->128 rows by SBUF->SBUF DMA broadcast in
                # 1024-col blocks, issued one block ahead of use so the
                # (slack-rich) scalar exp never waits on the transfer.
                nblk = NCH // 2
                border = list(range(nblk - 1, -1, -1)) if rev else list(range(nblk))

                def issue_del(j, blk):
                    bsl = slice(blk * 2 * TC, (blk + 1) * 2 * TC)
                    rsl = slice(gr0 + j * 8, gr0 + j * 8 + 8)
                    dr = scsb.tile([128, 2 * TC], BF16, tag=f"del_rep{j}", bufs=2)
                    nc.sync.dma_start(
                        dr[:],
                        del_sb[hh][rsl, bsl]
                        .rearrange("p (one t) -> p one t", one=1)
                        .broadcast_to([8, 16, 2 * TC]),
                    )
                    return dr

                del_cur = {j: issue_del(j, border[0]) for j in range(GDB)}
                del_nxt = {}

                for it in range(NCH):
                    c = (NCH - 1 - it) if rev else it
                    tsl = slice(c * TC, (c + 1) * TC)
                    pair = it // 2
                    if it % 2 == 0:
                        if pair > 0:
                            del_cur = del_nxt
                        if pair + 1 < nblk:
                            del_nxt = {j: issue_del(j, border[pair + 1])
                                       for j in range(GDB)}

                    blk = border[pair]
                    sub = c - blk * 2

                    brs = b_rep[:, tsl]
                    crs = c_rep[:, tsl]

                    psY = scpsy.tile([128, TC], F32, tag="psY")

                    for j in range(GDB):
                        db = g * GDB + j

                        dA = scsb.tile([128, TC], BF16, tag="dA", bufs=3)
                        nc.scalar.activation(
                            dA[:],
                            del_cur[j][:, sub * TC : (sub + 1) * TC],
                            AF.Exp, scale=a_dn[:, db : db + 1],
                        )

                        ps_u = scps.tile([128, TC], F32, tag="ps_u", bufs=2)
                        with tc.high_priority(offset=80):
                            nc.tensor.matmul(
                                ps_u[:], r64[j][gp, :], du_sb[hh][gp, tsl],
                                start=True, stop=True,
                            )
                        # scalar drains PSUM to bf16 so the DVE multiply can
                        # run in its packed 2x mode (PSUM operands are 1x).
                        dur = scsb.tile([128, TC], BF16, tag="dur", bufs=3)
                        nc.scalar.copy(dur[:], ps_u[:])
                        dBu = scsb.tile([128, TC], BF16, tag="dBu", bufs=3)
                        nc.vector.tensor_tensor(dBu[:], dur[:], brs, AL.mult)

                        h = hpool.tile([128, TC], BF16, tag=f"h{j}", bufs=2)
                        if it == 0:
                            init = 0.0
                        elif rev:
                            init = h_prev[j][:, 0:1]
                        else:
                            init = h_prev[j][:, TC - 1 : TC]
                        if rev:
                            nc.vector.tensor_tensor_scan(
                                _rev(h[:]), _rev(dA[:]), _rev(dBu[:]), init,
                                AL.mult, AL.add,
                            )
                        else:
                            nc.vector.tensor_tensor_scan(
                                h[:], dA[:], dBu[:], init, AL.mult, AL.add
                            )
                        h_prev[j] = h

                        o = scsb.tile([128, TC], BF16, tag="o", bufs=4)
                        if j % 2 == 0:
                            nc.gpsimd.tensor_tensor(o[:], h[:], crs, AL.mult)
                        else:
                            nc.vector.tensor_tensor(o[:], h[:], crs, AL.mult)

                        nc.tensor.matmul(
                            psY[gp, :], rt64[j][:], o[:],
                            start=(j == 0), stop=(j == GDB - 1),
                        )

                    ysl = y_sb[hh][gp, tsl]
                    if rev:
                        nc.vector.tensor_tensor(ysl, ysl, psY[gp, :], AL.add)
                    else:
                        nc.scalar.copy(ysl, psY[gp, :])

    # ================= phase 5: D*u, gate with silu(z), out-proj =================
    with (
        tc.tile_pool(name="p6ps", bufs=2, space="PSUM") as p6ps,
        tc.tile_pool(name="p6sb", bufs=3) as p6sb,
    ):
        # descending: chunk 7's y_sb is finalized first by the rev direction
        for ch in range(NCH - 1, -1, -1):
            tsl = slice(ch * TC, (ch + 1) * TC)
            yg = []
            for hh in range(2):
                ps_z = p6ps.tile([HS[hh], TC], F32, tag=f"ps_z{hh}", bufs=1)
                nc.tensor.matmul(
                    ps_z[:],
                    w_int[:, 192 + HOF[hh] : 192 + HOF[hh] + HS[hh]],
                    xT[:, tsl],
                    start=True,
                    stop=True,
                )
                z_act = p6sb.tile([HS[hh], TC], F32, tag=f"z_act{hh}")
                nc.scalar.activation(z_act[:], ps_z[:], AF.Silu)

                yf = p6sb.tile([HS[hh], TC], F32, tag=f"yf{hh}")
                nc.vector.scalar_tensor_tensor(
                    yf[:], xc[hh][:, tsl], d2[hh][:, 0:1], y_sb[hh][:, tsl],
                    AL.mult, AL.add,
                )
                g = p6sb.tile([HS[hh], TC], BF16, tag=f"yg{hh}")
                nc.vector.tensor_tensor(g[:], yf[:], z_act[:], AL.mult)
                yg.append(g)

            for mt in range(TC // 128):
                msl = slice(mt * 128, (mt + 1) * 128)
                ps_o = p6ps.tile([128, 96], F32, tag="ps_o")
                nc.tensor.matmul(ps_o[:], yg[0][:, msl], wout_t[0][:], start=True, stop=False)
                nc.tensor.matmul(ps_o[:], yg[1][:, msl], wout_t[1][:], start=False, stop=True)
                stg = p6sb.tile([128, 96], F32, tag="stg")
                nc.scalar.copy(stg[:], ps_o[:])
                nc.sync.dma_start(
                    io["out"][ch * TC + mt * 128 : ch * TC + (mt + 1) * 128, :], stg[:]
                )


# ---------------------------------------------------------------------------
# host-side wrapper
# ---------------------------------------------------------------------------

def _host_constants(A_logs):
    import ml_dtypes

    A = -np.exp(np.asarray(A_logs, np.float32))  # (192, 16)
    p = np.arange(128)
    a_dn = np.zeros((128, NDB), np.float32)
    for db in range(NDB):
        a_dn[:, db] = A[db * 8 + p // 16, p % 16]
    r64 = np.zeros((GDB, 128, 128), np.float32)
    rt64 = np.zeros((GDB, 128, 64), np.float32)
    for j in range(GDB):
        r64[j] = (np.arange(128) % 64)[:, None] == (j * 8 + p // 16)[None, :]
        rt64[j] = (j * 8 + p // 16)[:, None] == np.arange(64)[None, :]
    ident = np.eye(128, dtype=np.float32)
    r64 = r64.astype(ml_dtypes.bfloat16)
    rt64 = rt64.astype(ml_dtypes.bfloat16)
    return a_dn, r64, rt64, ident


_NC_CACHE = {}


def _get_nc():
    if "nc" in _NC_CACHE:
        return _NC_CACHE["nc"]
    nc = bacc.Bacc(
        "TRN2", target_bir_lowering=False, debug=False, enable_asserts=False,
        num_devices=8,
    )
    io = {
        "x": nc.dram_tensor("x", [L, CM], F32, kind="ExternalInput").ap(),
        "w_in": nc.dram_tensor("w_in", [CM, 2 * D], BF16, kind="ExternalInput").ap(),
        "conv_w": nc.dram_tensor("conv_w", [3, 3, D, D], BF16, kind="ExternalInput").ap(),
        "conv_b": nc.dram_tensor("conv_b", [D], F32, kind="ExternalInput").ap(),
        "xpb_w": nc.dram_tensor("xpb_w", [D, 128], BF16, kind="ExternalInput").ap(),
        "xpc_w": nc.dram_tensor("xpc_w", [D, 128], BF16, kind="ExternalInput").ap(),
        "dtw_full": nc.dram_tensor("dtw_full", [D, D], BF16, kind="ExternalInput").ap(),
        "dt_proj_b": nc.dram_tensor("dt_proj_b", [D], F32, kind="ExternalInput").ap(),
        "d2": nc.dram_tensor("d2", [D], F32, kind="ExternalInput").ap(),
        "w_out": nc.dram_tensor("w_out", [D, CM], BF16, kind="ExternalInput").ap(),
        "a_dn": nc.dram_tensor("a_dn", [128, NDB], F32, kind="ExternalInput").ap(),
        "r64": nc.dram_tensor("r64", [GDB, 128, 128], BF16, kind="ExternalInput").ap(),
        "rt64": nc.dram_tensor("rt64", [GDB, 128, 64], BF16, kind="ExternalInput").ap(),
        "ident": nc.dram_tensor("ident", [128, 128], F32, kind="ExternalInput").ap(),
        "out": nc.dram_tensor("out", [L, CM], F32, kind="ExternalOutput").ap(),
    }
    with tile.TileContext(nc) as tc:
        with ExitStack() as ctx:
            build_kernel(ctx, tc, io)
    nc.compile()
    _NC_CACHE["nc"] = nc
    _NC_CACHE["io_names"] = list(io.keys())
    return nc


def make_in_maps(x, W_in, conv_w, conv_b, x_proj_w, dt_proj_w, dt_proj_b, A_logs,
                 Ds, W_out):
    import ml_dtypes

    f = lambda a: np.ascontiguousarray(np.asarray(a, dtype=np.float32))
    bf = lambda a: np.ascontiguousarray(np.asarray(a).astype(ml_dtypes.bfloat16))
    a_dn, r64, rt64, ident = _host_constants(A_logs)
    xpw = f(x_proj_w)
    common = {
        "w_in": bf(np.asarray(W_in).T), "conv_b": f(conv_b),
        "xpb_w": bf(np.tile(xpw[RK : RK + N], (8, 1)).T),
        "xpc_w": bf(np.tile(xpw[RK + N : RK + 2 * N], (8, 1)).T),
        "dtw_full": bf((f(dt_proj_w) @ xpw[:RK]).T),
        "dt_proj_b": f(dt_proj_b),
        "d2": f(Ds) * 2.0, "w_out": bf(np.asarray(W_out).T), "a_dn": a_dn,
        "r64": r64, "rt64": rt64, "ident": ident,
    }
    x = f(x)
    cw = f(conv_w)
    cw_t = np.ascontiguousarray(cw.transpose(0, 1, 3, 2))
    in_maps = []
    for c in range(8):
        b, lay = c // 2, c % 2
        xv = x[b] if lay == 0 else np.ascontiguousarray(x[b].transpose(1, 0, 2))
        in_maps.append(
            {**common, "x": xv.reshape(L, CM),
             "conv_w": bf((cw if lay == 0 else cw_t).transpose(2, 3, 1, 0))}
        )
    return in_maps


def assemble(parts):
    out = np.zeros((B, L, CM), np.float32)
    for c in range(8):
        b, lay = c // 2, c % 2
        p = parts[c]
        if lay:
            p = p.reshape(W, H, CM).transpose(1, 0, 2).reshape(L, CM)
        out[b] += p
    return out.reshape(B, H, W, CM)


def kernel(**inputs):
    from concourse.bass_utils import run_bass_kernel_spmd

    nc = _get_nc()
    in_maps = make_in_maps(**inputs)
    res = run_bass_kernel_spmd(nc, in_maps, list(range(8)))
    return assemble([res.results[c]["out"] for c in range(8)])



# revision 46
# speedup vs baseline: 1.1206x; 1.1206x over previous
"""BiMamba2D (VMamba-style 4-direction selective scan) Trainium2 Bass kernel.

Sharding: 8 cores = 4 batches x 2 scan layouts (hw / wh).  The wh layout is
realized by host-transposing the input image (and swapping the conv kernel's
spatial taps), so every core runs the same SPMD program.  Each core computes
both time directions (forward + reversed APs) of its layout and emits a
partial (L, 96) output; the host sums partials (gating and the output
projection are linear across the four direction contributions).

Scan-state layout: d-blocks of 8 channels x 16 states = 128 partitions
(row p of a d-block tile holds channel db*8 + p//16, state p%16).  The
recurrence runs as one tensor_tensor_scan per (d-block, time-chunk).
The 192 inner channels are split as 128 + 64 rows so every partition
offset is quad-aligned (0/64), which the engines require.

Performance notes (2.49 ms -> 0.98 ms on HW):
- All matmul operands are bf16 (fp32 matmuls run as 2 half-rate PE passes;
  bf16 is 1 full-rate pass).  Weights are host-transposed so every DMA is a
  dense row read (strided 2-byte gathers cost ~30k descriptors otherwise).
- delta/du are direction-independent: computed once (not per direction) in
  phase 3, bf16.  Exp/Ln are batched per chunk-pair so the activation table
  reloads a handful of times instead of per chunk (1.3 us per reload).
- delta is replicated 8->128 rows by SBUF->SBUF DMA broadcast, issued one
  1024-col block ahead; its consumer (scalar Exp) has slack to absorb DMA
  jitter.  du replication stays on the PE (DMA can't feed both: the 16x
  write amplification saturates the 8 HWDGE queues).
- The reversed direction is realized by giving tensor_tensor_scan reversed
  APs (step -1 on data0/data1/out); everything upstream and downstream
  stays in forward order with aligned fast DVE modes.
- h-state carry chains chunk-to-chunk via per-j h tiles read directly by the
  next scan (a scalar-engine carry copy adds a cross-engine hop that stalls
  the DVE).
- o = h*C runs on gpsimd for even j and the DVE for odd j: the split breaks
  a DVE<->gpsimd<->PE(psY accumulate) semaphore convoy that otherwise
  stalls ~20% of scans.
- Phases 1-3 (transpose, in-proj, conv, projections) are emitted interleaved
  per chunk with coexisting PSUM pools (8 banks exactly), and x/ident DMAs
  are priority-hoisted, so the scan phase starts at ~175 us instead of ~205.
- The DVE is the end-to-end bottleneck: scans are ~2.2 cyc/elem (feedback
  bubble), dBu is 1x (PSUM operand), ~96% DVE occupancy in the scan window.
"""

import os
import sys
from contextlib import ExitStack

import numpy as np

for _p in ("/opt/trn_rl_repo",):
    if _p not in sys.path and os.path.isdir(_p):
        sys.path.append(_p)

import concourse.bass as bass
import concourse.tile as tile
from concourse import bacc, mybir

F32 = mybir.dt.float32
F32R = mybir.dt.float32r
BF16 = mybir.dt.bfloat16
AL = mybir.AluOpType
AF = mybir.ActivationFunctionType

# Problem constants
B, H, W, CM = 4, 64, 64, 96
L = H * W  # 4096
D = 192  # d_inner
N = 16  # d_state
RK = 6  # dt_rank
TC = 512  # time-chunk
NCH = L // TC  # 8
NDB = D // 8  # 24 d-blocks
NG = 3  # groups of 64 channels
GDB = NDB // NG  # 8 d-blocks per group
HS = [128, 64]  # d_inner row split
HOF = [0, 128]  # absolute channel offset per half
# group -> (half index, row offset within half)
GMAP = [(0, 0), (0, 64), (1, 0)]
WP = W + 2  # padded row stride for conv


def _rev(ap):
    """Reverse an AP along its last (free) dim."""
    return ap[:, ::-1]


def build_kernel(ctx: ExitStack, tc: "tile.TileContext", io: dict):
    nc = tc.nc


    # ---------------- weight / constant loads ----------------
    wpool = ctx.enter_context(tc.tile_pool(name="wpool", bufs=1))

    w_int = wpool.tile([96, 384], BF16, name="w_int")
    nc.sync.dma_start(w_int[:], io["w_in"][:])

    # B/C projections with 16->128 row replication folded in (host-tiled),
    # and the dt projection folded through x_proj (host-matmul'd).
    xpb_t, xpc_t, dtw_t = [], [], []
    for hh in range(2):
        hsl = slice(HOF[hh], HOF[hh] + HS[hh])
        t = wpool.tile([HS[hh], 128], BF16, name=f"xpb_t{hh}")
        nc.sync.dma_start(t[:], io["xpb_w"][hsl, :])
        xpb_t.append(t)
        t = wpool.tile([HS[hh], 128], BF16, name=f"xpc_t{hh}")
        nc.sync.dma_start(t[:], io["xpc_w"][hsl, :])
        xpc_t.append(t)
        t = wpool.tile([HS[hh], 192], BF16, name=f"dtw_t{hh}")
        nc.sync.dma_start(t[:], io["dtw_full"][hsl, :])
        dtw_t.append(t)

    wout_t = []
    for hh in range(2):
        t = wpool.tile([HS[hh], 96], BF16, name=f"wout_t{hh}")
        nc.sync.dma_start(
            t[:], io["w_out"][HOF[hh] : HOF[hh] + HS[hh], :]
        )
        wout_t.append(t)

    def vec_col(name):
        tiles = []
        for hh in range(2):
            t = wpool.tile([HS[hh], 1], F32, name=f"{name}{hh}")
            nc.sync.dma_start(
                t[:],
                io[name][HOF[hh] : HOF[hh] + HS[hh]].rearrange("(p one) -> p one", one=1),
            )
            tiles.append(t)
        return tiles

    dtb = vec_col("dt_proj_b")
    convb = vec_col("conv_b")
    d2 = vec_col("d2")

    a_dn = wpool.tile([128, NDB], F32, name="a_dn")
    nc.sync.dma_start(a_dn[:], io["a_dn"][:])
    # r64 rows are duplicated (0..63 == 64..127) so the lhsT slice can sit
    # at the same base partition as its rhs (a group-base requirement).
    r64 = []  # [j]: [128, 128]; rows k: (k%64 == j*8 + p//16)
    rt64 = []  # [j]: [128, 64] n-contraction lhsT into rows j*8..j*8+8
    for j in range(GDB):
        t = wpool.tile([128, 128], BF16, name=f"r64_{j}")
        nc.sync.dma_start(t[:], io["r64"][j])
        r64.append(t)
        t2 = wpool.tile([128, 64], BF16, name=f"rt64_{j}")
        nc.sync.dma_start(t2[:], io["rt64"][j])
        rt64.append(t2)
    ident = wpool.tile([128, 128], F32, name="ident")
    with tc.high_priority():
        nc.sync.dma_start(ident[:], io["ident"][:])

    # ---------------- persistent big buffers ----------------
    ppool = ctx.enter_context(tc.tile_pool(name="persist", bufs=1))
    xT = ppool.tile([96, L], BF16, name="xT")  # x transposed (ch, t)
    xc = [ppool.tile([HS[hh], L], BF16, name=f"xc{hh}") for hh in range(2)]
    y_sb = [ppool.tile([HS[hh], L], F32, name=f"y{hh}") for hh in range(2)]
    b_rep = ppool.tile([128, L], BF16, name="b_rep")
    c_rep = ppool.tile([128, L], BF16, name="c_rep")

    # del/du/e1 + phase-3 PSUM live past the conv, so enter them below the
    # conv-scoped pools on the pool stack.
    dpool = ctx.enter_context(tc.tile_pool(name="dpool", bufs=1))
    del_sb = [dpool.tile([HS[hh], L], BF16, name=f"del{hh}") for hh in range(2)]
    du_sb = [dpool.tile([HS[hh], L], BF16, name=f"du{hh}") for hh in range(2)]
    p3ps = ctx.enter_context(tc.tile_pool(name="p3ps", bufs=2, space="PSUM"))
    e1pool = ctx.enter_context(tc.tile_pool(name="e1pool", bufs=1))
    e1_sb = [e1pool.tile([HS[hh], L], BF16, name=f"e1_{hh}") for hh in range(2)]

    # ================= phase 1: transpose x + input projection =================
    with (
        tc.tile_pool(name="padpool", bufs=1) as padpool,
        tc.tile_pool(name="cwpool", bufs=1) as cwpool,
    ):
        # conv weights: lhsT [d_in HS[ih], d_out HS[oh]] per (ih, oh, kh, kw)
        cw = {}
        for ih in range(2):
            for oh in range(2):
                for kh in range(3):
                    for kw in range(3):
                        t = cwpool.tile([HS[ih], HS[oh]], BF16, name=f"cw{ih}{oh}{kh}{kw}")
                        src = io["conv_w"][
                            kh,
                            kw,
                            HOF[ih] : HOF[ih] + HS[ih],
                            HOF[oh] : HOF[oh] + HS[oh],
                        ]
                        nc.sync.dma_start(t[:], src)
                        cw[(ih, oh, kh, kw)] = t

        xp_pad = [
            padpool.tile([HS[hh], (H + 2) * WP], BF16, name=f"xp_pad{hh}")
            for hh in range(2)
        ]
        for hh in range(2):
            nc.gpsimd.memset(xp_pad[hh][:], 0.0)

        with (
            tc.tile_pool(name="p1sb", bufs=3) as p1sb,
            tc.tile_pool(name="p1ps", bufs=2, space="PSUM") as p1ps,
        ):
            # x tiles + transposes are the critical path at kernel start;
            # hoist them above the (deferred-use) weight DMAs.
            with tc.high_priority():
                for m in range(L // 128):
                    xt = p1sb.tile([128, 96], F32, tag="xt")
                    nc.sync.dma_start(xt[:], io["x"][m * 128 : (m + 1) * 128, :])
                    ps_t = p1ps.tile([96, 128], F32, tag="ps_t")
                    nc.tensor.transpose(ps_t[:], xt[:], ident[:])
                    nc.scalar.copy(xT[:, m * 128 : (m + 1) * 128], ps_t[:])

            for ch in range(NCH):
                tsl = slice(ch * TC, (ch + 1) * TC)
                for oh in range(2):
                    ps = p1ps.tile([HS[oh], TC], F32, tag=f"ps_ip{oh}", bufs=1)
                    nc.tensor.matmul(
                        ps[:],
                        w_int[:, HOF[oh] : HOF[oh] + HS[oh]],
                        xT[:, tsl],
                        start=True,
                        stop=True,
                    )
                    # write into padded conv buffer rows [ch*8+1..ch*8+8], cols 1..64
                    dst = (
                        xp_pad[oh][:]
                        .rearrange("p (h w) -> p h w", w=WP)[
                            :, ch * 8 + 1 : ch * 8 + 9, 1 : W + 1
                        ]
                    )
                    nc.scalar.copy(dst, ps[:])

        # ========== phase 2+3 interleaved per chunk pair: 3x3 conv ==========
        # + bias/silu, then immediately B/C/dt projections and delta for the
        # same chunk, so the scan phase can start long before the last conv
        # chunk finishes.  Exp/Ln batched per pair to limit ACT table reloads.
        with tc.tile_pool(name="p2ps", bufs=2, space="PSUM") as p2ps:
            for pb in range(0, NCH, 2):
                for ch in (pb, pb + 1):
                    tsl = slice(ch * TC, (ch + 1) * TC)
                    for oh in range(2):
                        ps = p2ps.tile([HS[oh], TC], F32, tag=f"ps_cv{oh}")
                        first = True
                        for ih in range(2):
                            for kh in range(3):
                                for kw in range(3):
                                    rhs = (
                                        xp_pad[ih][:]
                                        .rearrange("p (h w) -> p h w", w=WP)[
                                            :, ch * 8 + kh : ch * 8 + kh + 8, kw : kw + W
                                        ]
                                    )
                                    last = ih == 1 and kh == 2 and kw == 2
                                    nc.tensor.matmul(
                                        ps[:],
                                        cw[(ih, oh, kh, kw)][:],
                                        rhs,
                                        start=first,
                                        stop=last,
                                    )
                                    first = False
                        nc.scalar.activation(
                            xc[oh][:, tsl], ps[:], AF.Silu, bias=convb[oh][:, 0:1]
                        )
                    ps_b = p3ps.tile([128, TC], F32, tag="ps_bc")
                    nc.tensor.matmul(ps_b[:], xpb_t[0][:], xc[0][:, tsl], start=True, stop=False)
                    nc.tensor.matmul(ps_b[:], xpb_t[1][:], xc[1][:, tsl], start=False, stop=True)
                    nc.scalar.copy(b_rep[:, tsl], ps_b[:])
                    ps_c = p3ps.tile([128, TC], F32, tag="ps_bc")
                    nc.tensor.matmul(ps_c[:], xpc_t[0][:], xc[0][:, tsl], start=True, stop=False)
                    nc.tensor.matmul(ps_c[:], xpc_t[1][:], xc[1][:, tsl], start=False, stop=True)
                    nc.scalar.copy(c_rep[:, tsl], ps_c[:])
                    for hh in range(2):
                        hsl = slice(HOF[hh], HOF[hh] + HS[hh])
                        ps_dt = p3ps.tile([HS[hh], TC], F32, tag=f"ps_dt{hh}", bufs=1)
                        nc.tensor.matmul(ps_dt[:], dtw_t[0][:, hsl], xc[0][:, tsl], start=True, stop=False)
                        nc.tensor.matmul(ps_dt[:], dtw_t[1][:, hsl], xc[1][:, tsl], start=False, stop=True)
                        nc.scalar.activation(
                            e1_sb[hh][:, tsl], ps_dt[:], AF.Exp, bias=dtb[hh][:, 0:1]
                        )
                for ch in (pb, pb + 1):
                    tsl = slice(ch * TC, (ch + 1) * TC)
                    for hh in range(2):
                        nc.scalar.activation(
                            del_sb[hh][:, tsl], e1_sb[hh][:, tsl], AF.Ln, bias=1.0
                        )
                        nc.vector.tensor_tensor(
                            du_sb[hh][:, tsl], del_sb[hh][:, tsl], xc[hh][:, tsl], AL.mult
                        )

    # ================= phase 4: selective scan (fwd + rev) =================
    # Everything is kept in forward (data) order; the time-reversed direction
    # is realized purely by giving tensor_tensor_scan reversed APs, so its
    # output h lands back in data order.  du is replicated 64->128 partitions
    # by an SBUF->SBUF DMA broadcast (8 src rows x16), which keeps dBu's
    # operands in SBUF/bf16 (fast DVE mode) and frees PE/PSUM.
    with (
        tc.tile_pool(name="scps", bufs=3, space="PSUM") as scps,
        tc.tile_pool(name="scpsy", bufs=2, space="PSUM") as scpsy,
        tc.tile_pool(name="scsb", bufs=2) as scsb,
        tc.tile_pool(name="hpool", bufs=1) as hpool,
    ):
        for rev in (0, 1):
            for g in range(NG):
                hh, gr0 = GMAP[g]
                h_prev = {}
                gp = slice(gr0, gr0 + 64)  # group's partition slice

                # delta replicated 8->128 rows by SBUF->SBUF DMA broadcast in
                # 1024-col blocks, issued one block ahead of use so the
                # (slack-rich) scalar exp never waits on the transfer.
                nblk = NCH // 2
                border = list(range(nblk - 1, -1, -1)) if rev else list(range(nblk))

                def issue_del(j, blk):
                    bsl = slice(blk * 2 * TC, (blk + 1) * 2 * TC)
                    rsl = slice(gr0 + j * 8, gr0 + j * 8 + 8)
                    dr = scsb.tile([128, 2 * TC], BF16, tag=f"del_rep{j}", bufs=2)
                    nc.sync.dma_start(
                        dr[:],
                        del_sb[hh][rsl, bsl]
                        .rearrange("p (one t) -> p one t", one=1)
                        .broadcast_to([8, 16, 2 * TC]),
                    )
                    return dr

                del_cur = {j: issue_del(j, border[0]) for j in range(GDB)}
                del_nxt = {}

                for it in range(NCH):
                    c = (NCH - 1 - it) if rev else it
                    tsl = slice(c * TC, (c + 1) * TC)
                    pair = it // 2
                    if it % 2 == 0:
                        if pair > 0:
                            del_cur = del_nxt
                        if pair + 1 < nblk:
                            del_nxt = {j: issue_del(j, border[pair + 1])
                                       for j in range(GDB)}

                    blk = border[pair]
                    sub = c - blk * 2

                    brs = b_rep[:, tsl]
                    crs = c_rep[:, tsl]

                    psY = scpsy.tile([128, TC], F32, tag="psY")

                    for j in range(GDB):
                        db = g * GDB + j

                        dA = scsb.tile([128, TC], BF16, tag="dA", bufs=3)
                        nc.scalar.activation(
                            dA[:],
                            del_cur[j][:, sub * TC : (sub + 1) * TC],
                            AF.Exp, scale=a_dn[:, db : db + 1],
                        )

                        ps_u = scps.tile([128, TC], F32, tag="ps_u", bufs=2)
                        with tc.high_priority(offset=80):
                            nc.tensor.matmul(
                                ps_u[:], r64[j][gp, :], du_sb[hh][gp, tsl],
                                start=True, stop=True,
                            )
                        dBu = scsb.tile([128, TC], BF16, tag="dBu", bufs=3)
                        nc.vector.scalar_tensor_tensor(
                            dBu[:], ps_u[:], 1.0, brs, AL.mult, AL.mult
                        )

                        h = hpool.tile([128, TC], BF16, tag=f"h{j}", bufs=2)
                        if it == 0:
                            init = 0.0
                        elif rev:
                            init = h_prev[j][:, 0:1]
                        else:
                            init = h_prev[j][:, TC - 1 : TC]
                        if rev:
                            nc.vector.tensor_tensor_scan(
                                _rev(h[:]), _rev(dA[:]), _rev(dBu[:]), init,
                                AL.mult, AL.add,
                            )
                        else:
                            nc.vector.tensor_tensor_scan(
                                h[:], dA[:], dBu[:], init, AL.mult, AL.add
                            )
                        h_prev[j] = h

                        o = scsb.tile([128, TC], BF16, tag="o", bufs=4)
                        if j % 2 == 0:
                            nc.gpsimd.tensor_tensor(o[:], h[:], crs, AL.mult)
                        else:
                            nc.vector.tensor_tensor(o[:], h[:], crs, AL.mult)

                        nc.tensor.matmul(
                            psY[gp, :], rt64[j][:], o[:],
                            start=(j == 0), stop=(j == GDB - 1),
                        )

                    ysl = y_sb[hh][gp, tsl]
                    if rev:
                        nc.vector.tensor_tensor(ysl, ysl, psY[gp, :], AL.add)
                    else:
                        nc.scalar.copy(ysl, psY[gp, :])

    # ================= phase 5: D*u, gate with silu(z), out-proj =================
    with (
        tc.tile_pool(name="p6ps", bufs=2, space="PSUM") as p6ps,
        tc.tile_pool(name="p6sb", bufs=3) as p6sb,
    ):
        # descending: chunk 7's y_sb is finalized first by the rev direction
        for ch in range(NCH - 1, -1, -1):
            tsl = slice(ch * TC, (ch + 1) * TC)
            yg = []
            for hh in range(2):
                ps_z = p6ps.tile([HS[hh], TC], F32, tag=f"ps_z{hh}", bufs=1)
                nc.tensor.matmul(
                    ps_z[:],
                    w_int[:, 192 + HOF[hh] : 192 + HOF[hh] + HS[hh]],
                    xT[:, tsl],
                    start=True,
                    stop=True,
                )
                z_act = p6sb.tile([HS[hh], TC], F32, tag=f"z_act{hh}")
                nc.scalar.activation(z_act[:], ps_z[:], AF.Silu)

                yf = p6sb.tile([HS[hh], TC], F32, tag=f"yf{hh}")
                nc.vector.scalar_tensor_tensor(
                    yf[:], xc[hh][:, tsl], d2[hh][:, 0:1], y_sb[hh][:, tsl],
                    AL.mult, AL.add,
                )
                g = p6sb.tile([HS[hh], TC], BF16, tag=f"yg{hh}")
                nc.vector.tensor_tensor(g[:], yf[:], z_act[:], AL.mult)
                yg.append(g)

            for mt in range(TC // 128):
                msl = slice(mt * 128, (mt + 1) * 128)
                ps_o = p6ps.tile([128, 96], F32, tag="ps_o")
                nc.tensor.matmul(ps_o[:], yg[0][:, msl], wout_t[0][:], start=True, stop=False)
                nc.tensor.matmul(ps_o[:], yg[1][:, msl], wout_t[1][:], start=False, stop=True)
                stg = p6sb.tile([128, 96], F32, tag="stg")
                nc.scalar.copy(stg[:], ps_o[:])
                nc.sync.dma_start(
                    io["out"][ch * TC + mt * 128 : ch * TC + (mt + 1) * 128, :], stg[:]
                )


# ---------------------------------------------------------------------------
# host-side wrapper
# ---------------------------------------------------------------------------

def _host_constants(A_logs):
    import ml_dtypes

    A = -np.exp(np.asarray(A_logs, np.float32))  # (192, 16)
    p = np.arange(128)
    a_dn = np.zeros((128, NDB), np.float32)
    for db in range(NDB):
        a_dn[:, db] = A[db * 8 + p // 16, p % 16]
    r64 = np.zeros((GDB, 128, 128), np.float32)
    rt64 = np.zeros((GDB, 128, 64), np.float32)
    for j in range(GDB):
        r64[j] = (np.arange(128) % 64)[:, None] == (j * 8 + p // 16)[None, :]
        rt64[j] = (j * 8 + p // 16)[:, None] == np.arange(64)[None, :]
    ident = np.eye(128, dtype=np.float32)
    r64 = r64.astype(ml_dtypes.bfloat16)
    rt64 = rt64.astype(ml_dtypes.bfloat16)
    return a_dn, r64, rt64, ident


_NC_CACHE = {}


def _get_nc():
    if "nc" in _NC_CACHE:
        return _NC_CACHE["nc"]
    nc = bacc.Bacc(
        "TRN2", target_bir_lowering=False, debug=False, enable_asserts=False,
        num_devices=8,
    )
    io = {
        "x": nc.dram_tensor("x", [L, CM], F32, kind="ExternalInput").ap(),
        "w_in": nc.dram_tensor("w_in", [CM, 2 * D], BF16, kind="ExternalInput").ap(),
        "conv_w": nc.dram_tensor("conv_w", [3, 3, D, D], BF16, kind="ExternalInput").ap(),
        "conv_b": nc.dram_tensor("conv_b", [D], F32, kind="ExternalInput").ap(),
        "xpb_w": nc.dram_tensor("xpb_w", [D, 128], BF16, kind="ExternalInput").ap(),
        "xpc_w": nc.dram_tensor("xpc_w", [D, 128], BF16, kind="ExternalInput").ap(),
        "dtw_full": nc.dram_tensor("dtw_full", [D, D], BF16, kind="ExternalInput").ap(),
        "dt_proj_b": nc.dram_tensor("dt_proj_b", [D], F32, kind="ExternalInput").ap(),
        "d2": nc.dram_tensor("d2", [D], F32, kind="ExternalInput").ap(),
        "w_out": nc.dram_tensor("w_out", [D, CM], BF16, kind="ExternalInput").ap(),
        "a_dn": nc.dram_tensor("a_dn", [128, NDB], F32, kind="ExternalInput").ap(),
        "r64": nc.dram_tensor("r64", [GDB, 128, 128], BF16, kind="ExternalInput").ap(),
        "rt64": nc.dram_tensor("rt64", [GDB, 128, 64], BF16, kind="ExternalInput").ap(),
        "ident": nc.dram_tensor("ident", [128, 128], F32, kind="ExternalInput").ap(),
        "out": nc.dram_tensor("out", [L, CM], F32, kind="ExternalOutput").ap(),
    }
    with tile.TileContext(nc) as tc:
        with ExitStack() as ctx:
            build_kernel(ctx, tc, io)
    nc.compile()
    _NC_CACHE["nc"] = nc
    _NC_CACHE["io_names"] = list(io.keys())
    return nc


def make_in_maps(x, W_in, conv_w, conv_b, x_proj_w, dt_proj_w, dt_proj_b, A_logs,
                 Ds, W_out):
    import ml_dtypes

    f = lambda a: np.ascontiguousarray(np.asarray(a, dtype=np.float32))
    bf = lambda a: np.ascontiguousarray(np.asarray(a).astype(ml_dtypes.bfloat16))
    a_dn, r64, rt64, ident = _host_constants(A_logs)
    xpw = f(x_proj_w)
    common = {
        "w_in": bf(np.asarray(W_in).T), "conv_b": f(conv_b),
        "xpb_w": bf(np.tile(xpw[RK : RK + N], (8, 1)).T),
        "xpc_w": bf(np.tile(xpw[RK + N : RK + 2 * N], (8, 1)).T),
        "dtw_full": bf((f(dt_proj_w) @ xpw[:RK]).T),
        "dt_proj_b": f(dt_proj_b),
        "d2": f(Ds) * 2.0, "w_out": bf(np.asarray(W_out).T), "a_dn": a_dn,
        "r64": r64, "rt64": rt64, "ident": ident,
    }
    x = f(x)
    cw = f(conv_w)
    cw_t = np.ascontiguousarray(cw.transpose(0, 1, 3, 2))
    in_maps = []
    for c in range(8):
        b, lay = c // 2, c % 2
        xv = x[b] if lay == 0 else np.ascontiguousarray(x[b].transpose(1, 0, 2))
        in_maps.append(
            {**common, "x": xv.reshape(L, CM),
             "conv_w": bf((cw if lay == 0 else cw_t).transpose(2, 3, 1, 0))}
        )
    return in_maps


def assemble(parts):
    out = np.zeros((B, L, CM), np.float32)
    for c in range(8):
        b, lay = c // 2, c % 2
        p = parts[c]
        if lay:
            p = p.reshape(W, H, CM).transpose(1, 0, 2).reshape(L, CM)
        out[b] += p
    return out.reshape(B, H, W, CM)


def kernel(**inputs):
    from concourse.bass_utils import run_bass_kernel_spmd

    nc = _get_nc()
    in_maps = make_in_maps(**inputs)
    res = run_bass_kernel_spmd(nc, in_maps, list(range(8)))
    return assemble([res.results[c]["out"] for c in range(8)])

